# revision 5
# baseline (speedup 1.0000x reference)
"""MoE (top-2, E=16, cap=512) Trainium2 Bass kernel, data-parallel over batch.

Contract: kernel(**inputs) takes the full fp32 inputs and returns the full
(B, T, C) fp32 output, distributing batch elements across the 8 NeuronCores.

Dispatch semantics note: on this platform the jax reference's capacity-
overflow scatter/gather resolves out-of-range positions by linearized
(e*cap+pos) addressing with opaque-but-deterministic collision winners, so
the dispatch table is extracted by replaying the reference's exact dispatch
program once on device; the heavy compute (expert GEMMs, SiLU, combine) runs
in the Bass kernel below.

Combine is interleaved with the expert loop: tokens are grouped by the last
expert their two source rows depend on ("stage"), each stage's gathers read
only o[0:(m+1)*CAP] so the Tile scheduler overlaps them with later experts'
GEMMs. Combined rows are written in stage order and un-permuted on host.
"""
import numpy as np
import ml_dtypes
from contextlib import ExitStack

E = 16
K = 2
B = 8
T = 4096
C = 1024
H = 512
CAP = 512
NSLOT = E * CAP  # 8192

_programs = {}  # caps tuple -> compiled nc


# ---------------------------------------------------------------------------
# Reference dispatch replay (must match reference.py's _dispatch verbatim so
# the per-op jitted programs are identical)
# ---------------------------------------------------------------------------

def _replay_tables(x, w_router):
    import jax
    import jax.numpy as jnp

    def _dispatch(xb, pb, cap):
        T_, C_ = xb.shape
        top_k_probs, ei = jax.lax.top_k(pb, K)
        flat = ei.swapaxes(0, 1).ravel()
        one_hot = jax.nn.one_hot(flat, E, dtype=jnp.int32)
        pos = jnp.cumsum(one_hot, axis=0) * one_hot
        pos = pos.reshape(K, T_, E).swapaxes(0, 1)
        pos = jnp.max(pos, axis=-1) - 1
        ei2 = flat.reshape(K, T_).swapaxes(0, 1)
        xr = jnp.repeat(xb, K, axis=0)
        buf = jnp.zeros((E, cap, C_), xb.dtype)
        buf = buf.at[ei2.ravel(), pos.ravel()].set(xr, mode="drop")
        return top_k_probs, pos, ei2, buf

    xj = jnp.asarray(x)
    g = jnp.einsum('btc,ce->bte', xj, jnp.asarray(w_router))
    gate_probs = jax.nn.softmax(g, axis=-1)
    tk, pos, ei2, buf = jax.vmap(
        lambda xb, pb: _dispatch(xb, pb, CAP))(xj, gate_probs)
    bufr = jnp.reshape(buf, (B, NSLOT, C))
    heads = np.asarray(bufr[:, :, :8])
    norms = np.asarray(jnp.linalg.norm(bufr, axis=2))
    tk = np.asarray(tk)
    pos = np.asarray(pos)
    ei2 = np.asarray(ei2)

    winner = np.full((B, NSLOT), -1, np.int64)
    for b in range(B):
        hm = {x[b, t, :8].tobytes(): t for t in range(T)}
        for s in range(NSLOT):
            if norms[b, s] < 1e-7:
                continue
            t = hm.get(heads[b, s].tobytes(), -1)
            if t < 0:
                raise RuntimeError(f"unmatched slot b{b} s{s}")
            winner[b, s] = t
    return tk, pos, ei2, winner


def _wrap_idx(flat):
    """(N,) int -> [128, N//16] int16: value for flat j at (j%16, j//16),
    replicated 8x along partitions for the SWDGE cores."""
    n = len(flat)
    w = np.ascontiguousarray(np.asarray(flat, np.int16).reshape(n // 16, 16).T)
    return np.ascontiguousarray(np.tile(w, (8, 1)))


# ---------------------------------------------------------------------------
# Bass program (identical on all 8 cores; one batch element per core).
# caps[m] = number of combine rows (multiple of 128) emitted after expert m.
# ---------------------------------------------------------------------------

def _build_program(caps):
    import concourse.bass as bass
    import concourse.mybir as mybir
    import concourse.tile as tile
    from concourse import bacc

    R = sum(caps)
    offs = np.concatenate([[0], np.cumsum(caps)]).astype(int)

    dt = mybir.dt
    nc = bacc.Bacc("TRN2", target_bir_lowering=False, debug=False,
                   enable_asserts=False, num_devices=8)

    xbf = nc.dram_tensor("xbf", (T, C), dt.bfloat16, kind="ExternalInput").ap()
    wg = nc.dram_tensor("wg", (E, C, H), dt.bfloat16, kind="ExternalInput").ap()
    wf = nc.dram_tensor("wf", (E, C, H), dt.bfloat16, kind="ExternalInput").ap()
    wp = nc.dram_tensor("wp", (E, H, C), dt.bfloat16, kind="ExternalInput").ap()
    didx = nc.dram_tensor("didx", (128, NSLOT // 16), dt.int16,
                          kind="ExternalInput").ap()
    cidxa = nc.dram_tensor("cidxa", (128, R // 16), dt.int16,
                           kind="ExternalInput").ap()
    cidxb = nc.dram_tensor("cidxb", (128, R // 16), dt.int16,
                           kind="ExternalInput").ap()
    p0d = nc.dram_tensor("p0", (128, R // 128), dt.float32,
                         kind="ExternalInput").ap()
    p1d = nc.dram_tensor("p1", (128, R // 128), dt.float32,
                         kind="ExternalInput").ap()
    yb = nc.dram_tensor("ybuf", (R, C), dt.bfloat16, kind="ExternalOutput").ap()
    o = nc.dram_tensor("o", (NSLOT, C), dt.bfloat16, kind="ExternalOutput").ap()

    with tile.TileContext(nc) as tc, ExitStack() as ctx:
        const = ctx.enter_context(tc.tile_pool(name="const", bufs=1))
        wpool = ctx.enter_context(tc.tile_pool(name="wpool", bufs=3))
        epool = ctx.enter_context(tc.tile_pool(name="epool", bufs=3))
        hpool = ctx.enter_context(tc.tile_pool(name="hpool", bufs=2))
        spool = ctx.enter_context(tc.tile_pool(name="spool", bufs=3))
        opool = ctx.enter_context(tc.tile_pool(name="opool", bufs=3))
        cpool = ctx.enter_context(tc.tile_pool(name="cpool", bufs=2))
        ypool = ctx.enter_context(tc.tile_pool(name="ypool", bufs=3))
        ps1 = ctx.enter_context(tc.tile_pool(name="ps1", bufs=3, space="PSUM"))
        ps2 = ctx.enter_context(tc.tile_pool(name="ps2", bufs=2, space="PSUM"))

        didx_sb = const.tile([128, NSLOT // 16], dt.int16)
        nc.sync.dma_start(didx_sb, didx)
        cidxa_sb = const.tile([128, R // 16], dt.int16)
        nc.sync.dma_start(cidxa_sb, cidxa)
        cidxb_sb = const.tile([128, R // 16], dt.int16)
        nc.sync.dma_start(cidxb_sb, cidxb)
        p0_sb = const.tile([128, R // 128], dt.float32)
        nc.sync.dma_start(p0_sb, p0d)
        p1_sb = const.tile([128, R // 128], dt.float32)
        nc.sync.dma_start(p1_sb, p1d)

        max_cap = max(caps) if caps else 0

        def combine_stage(m):
            n = caps[m]
            if n == 0:
                return
            off = int(offs[m])
            osrc = o[: (m + 1) * CAP, :]
            nb = n // 128
            ga = cpool.tile([128, max_cap // 128, C], dt.bfloat16, tag="ga")
            nc.gpsimd.dma_gather(
                ga[:, :nb, :], osrc, cidxa_sb[:, off // 16:(off + n) // 16],
                n, n, C, transpose=False)
            gb = cpool.tile([128, max_cap // 128, C], dt.bfloat16, tag="gb")
            nc.gpsimd.dma_gather(
                gb[:, :nb, :], osrc, cidxb_sb[:, off // 16:(off + n) // 16],
                n, n, C, transpose=False)
            for blk in range(nb):
                col = off // 128 + blk
                ya = ypool.tile([128, C], dt.float32, tag="ya")
                yb2 = ypool.tile([128, C], dt.float32, tag="yb2")
                yout = ypool.tile([128, C], dt.bfloat16, tag="yout")
                nc.vector.tensor_scalar_mul(ya, ga[:, blk, :],
                                            p0_sb[:, col:col + 1])
                nc.scalar.activation(yb2, gb[:, blk, :],
                                     mybir.ActivationFunctionType.Copy,
                                     scale=p1_sb[:, col:col + 1])
                nc.vector.tensor_tensor(yout, ya, yb2, mybir.AluOpType.add)
                nc.sync.dma_start(yb[off + blk * 128:off + (blk + 1) * 128, :], yout)

        for e in range(E):
            eint = epool.tile([128, 8, CAP], dt.bfloat16, tag="eint")
            nc.gpsimd.dma_gather(eint, xbf, didx_sb[:, 32 * e:32 * (e + 1)],
                                 CAP, CAP, C, transpose=True)
            wg_sb = wpool.tile([128, 8, H], dt.bfloat16, tag="wg")
            nc.sync.dma_start(wg_sb, wg[e].rearrange("(cc p) h -> p cc h", p=128))
            wf_sb = wpool.tile([128, 8, H], dt.bfloat16, tag="wf")
            nc.sync.dma_start(wf_sb, wf[e].rearrange("(cc p) h -> p cc h", p=128))
            wp_sb = wpool.tile([128, 4, C], dt.bfloat16, tag="wp")
            nc.sync.dma_start(wp_sb, wp[e].rearrange("(hc p) c -> p hc c", p=128))

            ht = hpool.tile([128, 4, CAP], dt.bfloat16, tag="ht")
            for hc in range(4):
                gg = ps1.tile([128, CAP], dt.float32, tag="gg")
                hh = ps1.tile([128, CAP], dt.float32, tag="hh")
                for cc in range(8):
                    nc.tensor.matmul(gg, wg_sb[:, cc, 128 * hc:128 * (hc + 1)],
                                     eint[:, cc, :], start=cc == 0, stop=cc == 7)
                for cc in range(8):
                    nc.tensor.matmul(hh, wf_sb[:, cc, 128 * hc:128 * (hc + 1)],
                                     eint[:, cc, :], start=cc == 0, stop=cc == 7)
                sil = spool.tile([128, CAP], dt.float32, tag="sil")
                nc.scalar.activation(sil, gg,
                                     mybir.ActivationFunctionType.Silu)
                nc.vector.tensor_tensor(ht[:, hc, :], sil, hh,
                                        mybir.AluOpType.mult)
            for sc in range(4):
                ob = opool.tile([128, C], dt.bfloat16, tag="ob")
                for c2 in range(2):
                    ops = ps2.tile([128, 512], dt.float32, tag="ops")
                    for hc in range(4):
                        nc.tensor.matmul(
                            ops, ht[:, hc, 128 * sc:128 * (sc + 1)],
                            wp_sb[:, hc, 512 * c2:512 * (c2 + 1)],
                            start=hc == 0, stop=hc == 3)
                    nc.vector.tensor_copy(ob[:, 512 * c2:512 * (c2 + 1)], ops)
                nc.sync.dma_start(o[e * CAP + sc * 128:e * CAP + (sc + 1) * 128, :], ob)

            combine_stage(e)

    nc.compile()
    return nc


def _get_program(caps):
    key = tuple(caps)
    if key not in _programs:
        _programs[key] = _build_program(key)
    return _programs[key]


# ---------------------------------------------------------------------------
# Host orchestration
# ---------------------------------------------------------------------------

def _prepare_inputs(x, w_router, w_fc, w_gate, w_proj):
    bf16 = ml_dtypes.bfloat16
    tk, pos, ei2, winner = _replay_tables(x, w_router)

    wgt = np.ascontiguousarray(w_gate.astype(bf16))
    wfc = np.ascontiguousarray(w_fc.astype(bf16))
    wpj = np.ascontiguousarray(w_proj.astype(bf16))

    # stage assignment must be common across cores (one compiled program):
    # use per-batch max row counts per stage, then pad to 128.
    L_all = ei2.astype(np.int64) * CAP + pos            # (B, T, K)
    Lc_all = np.minimum(L_all, NSLOT - 1)
    stage_all = Lc_all.max(axis=2) // CAP               # (B, T)
    counts = np.zeros((B, E), np.int64)
    for b in range(B):
        counts[b] = np.bincount(stage_all[b], minlength=E)
    caps = tuple(int(-(-c // 128) * 128) for c in counts.max(axis=0))
    offs = np.concatenate([[0], np.cumsum(caps)]).astype(int)
    R = int(offs[-1])

    in_maps = []
    patches = []
    perms = np.zeros((B, T), np.int64)  # token t -> ybuf row
    gL_all = np.zeros((B, T, K), np.int64)
    for b in range(B):
        didx_flat = np.maximum(winner[b], 0)
        gL_all[b] = b * NSLOT + L_all[b]
        cross = np.where(L_all[b] >= NSLOT)
        for t, k in zip(*cross):
            patches.append((b, int(t)))

        ca = np.zeros(R, np.int64)
        cb = np.zeros(R, np.int64)
        pa = np.zeros(R, np.float32)
        pb = np.zeros(R, np.float32)
        fill = offs[:-1].copy()
        for t in range(T):
            r = fill[stage_all[b, t]]
            fill[stage_all[b, t]] += 1
            perms[b, t] = r
            ca[r] = Lc_all[b, t, 0]
            cb[r] = Lc_all[b, t, 1]
            pa[r] = tk[b, t, 0]
            pb[r] = tk[b, t, 1]

        in_maps.append({
            "xbf": np.ascontiguousarray(x[b].astype(bf16)),
            "wg": wgt, "wf": wfc, "wp": wpj,
            "didx": _wrap_idx(didx_flat),
            "cidxa": _wrap_idx(ca),
            "cidxb": _wrap_idx(cb),
            "p0": np.ascontiguousarray(pa.reshape(R // 128, 128).T),
            "p1": np.ascontiguousarray(pb.reshape(R // 128, 128).T),
        })
    return in_maps, tk, gL_all, sorted(set(patches)), perms, caps


def _run(in_maps, caps, trace=False):
    from concourse.bass_utils import run_bass_kernel_spmd
    nc = _get_program(caps)
    res = run_bass_kernel_spmd(nc, in_maps, core_ids=list(range(B)),
                               trace=trace)
    return res


def kernel(x, w_router, w_fc, w_gate, w_proj, _trace=False, _ret_extra=False):
    in_maps, tk, gL_all, patches, perms, caps = _prepare_inputs(
        np.asarray(x, np.float32), np.asarray(w_router, np.float32),
        np.asarray(w_fc, np.float32), np.asarray(w_gate, np.float32),
        np.asarray(w_proj, np.float32))
    res = _run(in_maps, caps, trace=_trace)
    outs = res.results
    y = np.stack([np.asarray(outs[b]["ybuf"]).astype(np.float32)[perms[b]]
                  for b in range(B)])
    if patches:
        of = np.stack([np.asarray(outs[b]["o"]) for b in range(B)])
        of = of.reshape(B * NSLOT, C).astype(np.float32)
        for (b, t) in patches:
            y[b, t] = (tk[b][t, 0] * of[gL_all[b, t, 0]]
                       + tk[b][t, 1] * of[gL_all[b, t, 1]])
    if _ret_extra:
        return y, res
    return y


if __name__ == "__main__":
    d = np.load("/root/problem/inputs.npz")
    y = kernel(**{k: d[k] for k in ("x", "w_router", "w_fc", "w_gate", "w_proj")})
    exp = np.load("/root/problem/expected.npy")
    rel = np.linalg.norm(y - exp) / np.linalg.norm(exp)
    print(f"rel_l2 = {rel:.4e}, absmax = {np.abs(y - exp).max():.4e}")


# revision 6
# speedup vs baseline: 1.0023x; 1.0023x over previous
"""MoE (top-2, E=16, cap=512) Trainium2 Bass kernel, data-parallel over batch.

Contract: kernel(**inputs) takes the full fp32 inputs and returns the full
(B, T, C) fp32 output, distributing batch elements across the 8 NeuronCores.

Dispatch semantics note: on this platform the jax reference's capacity-
overflow scatter/gather resolves out-of-range positions by linearized
(e*cap+pos) addressing with opaque-but-deterministic collision winners, so
the dispatch table is extracted by replaying the reference's exact dispatch
program once on device; the heavy compute (expert GEMMs, SiLU, combine) runs
in the Bass kernel below.

Combine is interleaved with the expert loop: tokens are grouped by the last
expert their two source rows depend on ("stage"), each stage's gathers read
only o[0:(m+1)*CAP] so the Tile scheduler overlaps them with later experts'
GEMMs. Combined rows are written in stage order and un-permuted on host.
"""
import numpy as np
import ml_dtypes
from contextlib import ExitStack

E = 16
K = 2
B = 8
T = 4096
C = 1024
H = 512
CAP = 512
NSLOT = E * CAP  # 8192

_programs = {}  # caps tuple -> compiled nc


# ---------------------------------------------------------------------------
# Reference dispatch replay (must match reference.py's _dispatch verbatim so
# the per-op jitted programs are identical)
# ---------------------------------------------------------------------------

def _replay_tables(x, w_router):
    import jax
    import jax.numpy as jnp

    def _dispatch(xb, pb, cap):
        T_, C_ = xb.shape
        top_k_probs, ei = jax.lax.top_k(pb, K)
        flat = ei.swapaxes(0, 1).ravel()
        one_hot = jax.nn.one_hot(flat, E, dtype=jnp.int32)
        pos = jnp.cumsum(one_hot, axis=0) * one_hot
        pos = pos.reshape(K, T_, E).swapaxes(0, 1)
        pos = jnp.max(pos, axis=-1) - 1
        ei2 = flat.reshape(K, T_).swapaxes(0, 1)
        xr = jnp.repeat(xb, K, axis=0)
        buf = jnp.zeros((E, cap, C_), xb.dtype)
        buf = buf.at[ei2.ravel(), pos.ravel()].set(xr, mode="drop")
        return top_k_probs, pos, ei2, buf

    xj = jnp.asarray(x)
    g = jnp.einsum('btc,ce->bte', xj, jnp.asarray(w_router))
    gate_probs = jax.nn.softmax(g, axis=-1)
    tk, pos, ei2, buf = jax.vmap(
        lambda xb, pb: _dispatch(xb, pb, CAP))(xj, gate_probs)
    bufr = jnp.reshape(buf, (B, NSLOT, C))
    heads = np.asarray(bufr[:, :, :8])
    norms = np.asarray(jnp.linalg.norm(bufr, axis=2))
    tk = np.asarray(tk)
    pos = np.asarray(pos)
    ei2 = np.asarray(ei2)

    winner = np.full((B, NSLOT), -1, np.int64)
    for b in range(B):
        hm = {x[b, t, :8].tobytes(): t for t in range(T)}
        for s in range(NSLOT):
            if norms[b, s] < 1e-7:
                continue
            t = hm.get(heads[b, s].tobytes(), -1)
            if t < 0:
                raise RuntimeError(f"unmatched slot b{b} s{s}")
            winner[b, s] = t
    return tk, pos, ei2, winner


def _wrap_idx(flat):
    """(N,) int -> [128, N//16] int16: value for flat j at (j%16, j//16),
    replicated 8x along partitions for the SWDGE cores."""
    n = len(flat)
    w = np.ascontiguousarray(np.asarray(flat, np.int16).reshape(n // 16, 16).T)
    return np.ascontiguousarray(np.tile(w, (8, 1)))


# ---------------------------------------------------------------------------
# Bass program (identical on all 8 cores; one batch element per core).
# caps[m] = number of combine rows (multiple of 128) emitted after expert m.
# ---------------------------------------------------------------------------

BNDS = [4 * (m + 1) for m in range(15)] + [61, 62, 63, 64]


def _build_program(caps):
    import concourse.bass as bass
    import concourse.mybir as mybir
    import concourse.tile as tile
    from concourse import bacc

    R = sum(caps)
    offs = np.concatenate([[0], np.cumsum(caps)]).astype(int)

    dt = mybir.dt
    nc = bacc.Bacc("TRN2", target_bir_lowering=False, debug=False,
                   enable_asserts=False, num_devices=8)

    xbf = nc.dram_tensor("xbf", (T, C), dt.bfloat16, kind="ExternalInput").ap()
    wg = nc.dram_tensor("wg", (E, C, H), dt.bfloat16, kind="ExternalInput").ap()
    wf = nc.dram_tensor("wf", (E, C, H), dt.bfloat16, kind="ExternalInput").ap()
    wp = nc.dram_tensor("wp", (E, H, C), dt.bfloat16, kind="ExternalInput").ap()
    didx = nc.dram_tensor("didx", (128, NSLOT // 16), dt.int16,
                          kind="ExternalInput").ap()
    cidxa = nc.dram_tensor("cidxa", (128, R // 16), dt.int16,
                           kind="ExternalInput").ap()
    cidxb = nc.dram_tensor("cidxb", (128, R // 16), dt.int16,
                           kind="ExternalInput").ap()
    p0d = nc.dram_tensor("p0", (128, R // 128), dt.float32,
                         kind="ExternalInput").ap()
    p1d = nc.dram_tensor("p1", (128, R // 128), dt.float32,
                         kind="ExternalInput").ap()
    yb = nc.dram_tensor("ybuf", (R, C), dt.bfloat16, kind="ExternalOutput").ap()
    o = nc.dram_tensor("o", (NSLOT, C), dt.bfloat16, kind="ExternalOutput").ap()

    with tile.TileContext(nc) as tc, ExitStack() as ctx:
        const = ctx.enter_context(tc.tile_pool(name="const", bufs=1))
        wpool = ctx.enter_context(tc.tile_pool(name="wpool", bufs=3))
        epool = ctx.enter_context(tc.tile_pool(name="epool", bufs=3))
        hpool = ctx.enter_context(tc.tile_pool(name="hpool", bufs=2))
        spool = ctx.enter_context(tc.tile_pool(name="spool", bufs=3))
        opool = ctx.enter_context(tc.tile_pool(name="opool", bufs=3))
        cpool = ctx.enter_context(tc.tile_pool(name="cpool", bufs=2))
        ypool = ctx.enter_context(tc.tile_pool(name="ypool", bufs=3))
        ps1 = ctx.enter_context(tc.tile_pool(name="ps1", bufs=3, space="PSUM"))
        ps2 = ctx.enter_context(tc.tile_pool(name="ps2", bufs=2, space="PSUM"))

        didx_sb = const.tile([128, NSLOT // 16], dt.int16)
        nc.sync.dma_start(didx_sb, didx)
        cidxa_sb = const.tile([128, R // 16], dt.int16)
        nc.sync.dma_start(cidxa_sb, cidxa)
        cidxb_sb = const.tile([128, R // 16], dt.int16)
        nc.sync.dma_start(cidxb_sb, cidxb)
        p0_sb = const.tile([128, R // 128], dt.float32)
        nc.sync.dma_start(p0_sb, p0d)
        p1_sb = const.tile([128, R // 128], dt.float32)
        nc.sync.dma_start(p1_sb, p1d)

        max_cap = max(caps) if caps else 0

        def combine_stage(m):
            n = caps[m]
            if n == 0:
                return
            off = int(offs[m])
            osrc = o[: BNDS[m] * 128, :]
            nb = n // 128
            ga = cpool.tile([128, max_cap // 128, C], dt.bfloat16, tag="ga")
            nc.gpsimd.dma_gather(
                ga[:, :nb, :], osrc, cidxa_sb[:, off // 16:(off + n) // 16],
                n, n, C, transpose=False)
            gb = cpool.tile([128, max_cap // 128, C], dt.bfloat16, tag="gb")
            nc.gpsimd.dma_gather(
                gb[:, :nb, :], osrc, cidxb_sb[:, off // 16:(off + n) // 16],
                n, n, C, transpose=False)
            for blk in range(nb):
                col = off // 128 + blk
                ya = ypool.tile([128, C], dt.float32, tag="ya")
                yb2 = ypool.tile([128, C], dt.float32, tag="yb2")
                yout = ypool.tile([128, C], dt.bfloat16, tag="yout")
                nc.vector.tensor_scalar_mul(ya, ga[:, blk, :],
                                            p0_sb[:, col:col + 1])
                nc.scalar.activation(yb2, gb[:, blk, :],
                                     mybir.ActivationFunctionType.Copy,
                                     scale=p1_sb[:, col:col + 1])
                nc.vector.tensor_tensor(yout, ya, yb2, mybir.AluOpType.add)
                nc.sync.dma_start(yb[off + blk * 128:off + (blk + 1) * 128, :], yout)

        for e in range(E):
            eint = epool.tile([128, 8, CAP], dt.bfloat16, tag="eint")
            nc.gpsimd.dma_gather(eint, xbf, didx_sb[:, 32 * e:32 * (e + 1)],
                                 CAP, CAP, C, transpose=True)
            wg_sb = wpool.tile([128, 8, H], dt.bfloat16, tag="wg")
            nc.sync.dma_start(wg_sb, wg[e].rearrange("(cc p) h -> p cc h", p=128))
            wf_sb = wpool.tile([128, 8, H], dt.bfloat16, tag="wf")
            nc.sync.dma_start(wf_sb, wf[e].rearrange("(cc p) h -> p cc h", p=128))
            wp_sb = wpool.tile([128, 4, C], dt.bfloat16, tag="wp")
            nc.sync.dma_start(wp_sb, wp[e].rearrange("(hc p) c -> p hc c", p=128))

            ht = hpool.tile([128, 4, CAP], dt.bfloat16, tag="ht")
            for hc in range(4):
                gg = ps1.tile([128, CAP], dt.float32, tag="gg")
                hh = ps1.tile([128, CAP], dt.float32, tag="hh")
                for cc in range(8):
                    nc.tensor.matmul(gg, wg_sb[:, cc, 128 * hc:128 * (hc + 1)],
                                     eint[:, cc, :], start=cc == 0, stop=cc == 7)
                for cc in range(8):
                    nc.tensor.matmul(hh, wf_sb[:, cc, 128 * hc:128 * (hc + 1)],
                                     eint[:, cc, :], start=cc == 0, stop=cc == 7)
                sil = spool.tile([128, CAP], dt.float32, tag="sil")
                nc.scalar.activation(sil, gg,
                                     mybir.ActivationFunctionType.Silu)
                nc.vector.tensor_tensor(ht[:, hc, :], sil, hh,
                                        mybir.AluOpType.mult)
            for sc in range(4):
                ob = opool.tile([128, C], dt.bfloat16, tag="ob")
                for c2 in range(2):
                    ops = ps2.tile([128, 512], dt.float32, tag="ops")
                    for hc in range(4):
                        nc.tensor.matmul(
                            ops, ht[:, hc, 128 * sc:128 * (sc + 1)],
                            wp_sb[:, hc, 512 * c2:512 * (c2 + 1)],
                            start=hc == 0, stop=hc == 3)
                    if c2 == 0:
                        nc.vector.tensor_copy(ob[:, 512 * c2:512 * (c2 + 1)], ops)
                    else:
                        nc.scalar.activation(
                            ob[:, 512 * c2:512 * (c2 + 1)], ops,
                            mybir.ActivationFunctionType.Copy)
                nc.sync.dma_start(o[e * CAP + sc * 128:e * CAP + (sc + 1) * 128, :], ob)
                nchunks = e * 4 + sc + 1
                for st, bnd in enumerate(BNDS):
                    if bnd == nchunks:
                        combine_stage(st)

    nc.compile()
    return nc


def _get_program(caps):
    key = tuple(caps)
    if key not in _programs:
        _programs[key] = _build_program(key)
    return _programs[key]


# ---------------------------------------------------------------------------
# Host orchestration
# ---------------------------------------------------------------------------

def _prepare_inputs(x, w_router, w_fc, w_gate, w_proj):
    bf16 = ml_dtypes.bfloat16
    tk, pos, ei2, winner = _replay_tables(x, w_router)

    wgt = np.ascontiguousarray(w_gate.astype(bf16))
    wfc = np.ascontiguousarray(w_fc.astype(bf16))
    wpj = np.ascontiguousarray(w_proj.astype(bf16))

    # stage assignment must be common across cores (one compiled program):
    # use per-batch max row counts per stage, then pad to 128.
    L_all = ei2.astype(np.int64) * CAP + pos            # (B, T, K)
    Lc_all = np.minimum(L_all, NSLOT - 1)
    need = Lc_all.max(axis=2) // 128 + 1                # (B, T) o-chunks needed
    bnds = np.asarray(BNDS)
    stage_all = np.searchsorted(bnds, need)             # first stage with bnd>=need
    S = len(BNDS)
    counts = np.zeros((B, S), np.int64)
    for b in range(B):
        counts[b] = np.bincount(stage_all[b], minlength=S)
    caps = tuple(int(-(-c // 128) * 128) for c in counts.max(axis=0))
    offs = np.concatenate([[0], np.cumsum(caps)]).astype(int)
    R = int(offs[-1])

    in_maps = []
    patches = []
    perms = np.zeros((B, T), np.int64)  # token t -> ybuf row
    gL_all = np.zeros((B, T, K), np.int64)
    for b in range(B):
        didx_flat = np.maximum(winner[b], 0)
        gL_all[b] = b * NSLOT + L_all[b]
        cross = np.where(L_all[b] >= NSLOT)
        for t, k in zip(*cross):
            patches.append((b, int(t)))

        ca = np.zeros(R, np.int64)
        cb = np.zeros(R, np.int64)
        pa = np.zeros(R, np.float32)
        pb = np.zeros(R, np.float32)
        fill = offs[:-1].copy()
        for t in range(T):
            r = fill[stage_all[b, t]]
            fill[stage_all[b, t]] += 1
            perms[b, t] = r
            ca[r] = Lc_all[b, t, 0]
            cb[r] = Lc_all[b, t, 1]
            pa[r] = tk[b, t, 0]
            pb[r] = tk[b, t, 1]

        in_maps.append({
            "xbf": np.ascontiguousarray(x[b].astype(bf16)),
            "wg": wgt, "wf": wfc, "wp": wpj,
            "didx": _wrap_idx(didx_flat),
            "cidxa": _wrap_idx(ca),
            "cidxb": _wrap_idx(cb),
            "p0": np.ascontiguousarray(pa.reshape(R // 128, 128).T),
            "p1": np.ascontiguousarray(pb.reshape(R // 128, 128).T),
        })
    return in_maps, tk, gL_all, sorted(set(patches)), perms, caps


def _run(in_maps, caps, trace=False):
    from concourse.bass_utils import run_bass_kernel_spmd
    nc = _get_program(caps)
    res = run_bass_kernel_spmd(nc, in_maps, core_ids=list(range(B)),
                               trace=trace)
    return res


def kernel(x, w_router, w_fc, w_gate, w_proj, _trace=False, _ret_extra=False):
    in_maps, tk, gL_all, patches, perms, caps = _prepare_inputs(
        np.asarray(x, np.float32), np.asarray(w_router, np.float32),
        np.asarray(w_fc, np.float32), np.asarray(w_gate, np.float32),
        np.asarray(w_proj, np.float32))
    res = _run(in_maps, caps, trace=_trace)
    outs = res.results
    y = np.stack([np.asarray(outs[b]["ybuf"]).astype(np.float32)[perms[b]]
                  for b in range(B)])
    if patches:
        of = np.stack([np.asarray(outs[b]["o"]) for b in range(B)])
        of = of.reshape(B * NSLOT, C).astype(np.float32)
        for (b, t) in patches:
            y[b, t] = (tk[b][t, 0] * of[gL_all[b, t, 0]]
                       + tk[b][t, 1] * of[gL_all[b, t, 1]])
    if _ret_extra:
        return y, res
    return y


if __name__ == "__main__":
    d = np.load("/root/problem/inputs.npz")
    y = kernel(**{k: d[k] for k in ("x", "w_router", "w_fc", "w_gate", "w_proj")})
    exp = np.load("/root/problem/expected.npy")
    rel = np.linalg.norm(y - exp) / np.linalg.norm(exp)
    print(f"rel_l2 = {rel:.4e}, absmax = {np.abs(y - exp).max():.4e}")
